# revision 2
# baseline (speedup 1.0000x reference)
"""Causal single-head attention on 8 TRN2 NeuronCores — v2.2.

Data-parallel over batch (64 per core), weights replicated.

Software pipeline, GRP=2 batches per group, one group stage per
iteration; every instruction's inputs are >=1 full iteration old:

  it = g+0: PE   qk chain (3 mm) -> qk[g%2]; v (12 mm) -> pair[g%2] spare
  it = g+1: DVE  q drain, v drain; Act k drain         -> SBUF
  it = g+2: PE   scores (4 mm)   -> pair[g%2] s-region
  it = g+3: Act  exp (one 768-col call, scaled)        -> et SBUF
  it = g+4: Pool causal mask (one affine_select)
  it = g+5: PE   AV + denominator (6 mm, ones column)  -> av[g%2]
  it = g+6: DVE  reciprocal + normalize-from-PSUM      -> ot; DMA out x2

PSUM is laid out so every bank is either PE-written or engine-read in
a given iteration, never both (avoids bank-hazard serialization and
makes dep-tracking granularity irrelevant):
  banks 0-3: pair0/pair1 tiles [128,2,512]f32 — per bank: scores batch
             slot [0:384) + v-projection spare [384:512)
  banks 4-5: qk0/qk1 [128,512]f32 (packed q|k projection)
  banks 6-7: av0/av1 [128,512]f32 (4 AV chains at 128-col pitch, 65
             cols each: 64 outputs + ones-column denominator)
Tiles alternate phases by group parity; drains/readers run exactly one
iteration behind the writers.
"""

import sys

for _p in ("/opt/trn_rl_repo",):
    if _p not in sys.path:
        sys.path.insert(0, _p)

import numpy as np
import ml_dtypes

import concourse.bass as bass
from concourse import bacc
import concourse.mybir as mybir
from concourse.tile import TileContext
from concourse.bass_utils import run_bass_kernel_spmd

B, S, E, H = 512, 256, 384, 64
NCORES = 8
BPC = B // NCORES
GRP = 2
NG = BPC // GRP            # 32 groups
SCALE = float(E) ** -0.5
EC = E // 128
PF = 4                     # x prefetch distance (groups, even)
XR = 8                     # x ring slots
VS = 8                     # v ring slots

BF16 = mybir.dt.bfloat16
F32 = mybir.dt.float32

_cache = {}


def build_nc():
    nc = bacc.Bacc()
    xt_d = nc.dram_tensor("xt", [128, NG, EC, GRP, S], BF16, kind="ExternalInput")
    wqk_d = nc.dram_tensor("wqk", [128, EC, 128], BF16, kind="ExternalInput")
    wv_d = nc.dram_tensor("wv", [128, EC, H], BF16, kind="ExternalInput")
    out_d = nc.dram_tensor("out", [128, NG, GRP * 2 * H], BF16, kind="ExternalOutput")

    EXP = mybir.ActivationFunctionType.Exp
    CPY = mybir.ActivationFunctionType.Copy

    with TileContext(nc) as tc:
        with (
            tc.tile_pool(name="wconst", bufs=1) as wpool,
            tc.tile_pool(name="qksb", bufs=3) as qk_pool,
            tc.tile_pool(name="et", bufs=4) as et_pool,
            tc.tile_pool(name="otp", bufs=1) as ot_pool,
            tc.tile_pool(name="ps", bufs=1, space="PSUM") as ps,
        ):
            # ---- persistent SBUF ----
            x_ring = wpool.tile([128, XR, EC, GRP, S], BF16)
            wqk_sb = wpool.tile([128, EC, 128], BF16)
            wv_sb = wpool.tile([128, EC, H], BF16)
            v_sb = wpool.tile([128, VS, GRP * 2, H + 1], BF16)
            ot_ring = ot_pool.tile([128, 8, GRP * 2, H], BF16)
            nc.vector.memset(v_sb, 1.0)

            # ---- PSUM: phase-alternating tiles (8 banks total) ----
            pair = [
                ps.tile([128, GRP, 512], F32, tag="pair0", name="pair0"),
                ps.tile([128, GRP, 512], F32, tag="pair1", name="pair1"),
            ]
            qk = [
                ps.tile([128, 512], F32, tag="qk0", name="qk0"),
                ps.tile([128, 512], F32, tag="qk1", name="qk1"),
            ]
            av = [
                ps.tile([128, 512], F32, tag="av0", name="av0"),
                ps.tile([128, 512], F32, tag="av1", name="av1"),
            ]

            # ---- prologue: weights + first x pairs ----
            nc.sync.dma_start(wqk_sb, wqk_d[:, :, :])
            nc.sync.dma_start(wv_sb, wv_d[:, :, :])
            for g0 in range(0, min(PF, NG), 2):
                nc.sync.dma_start(
                    x_ring[:, g0:g0 + 2], xt_d[:, g0:g0 + 2])

            qts = [None] * NG
            kts = [None] * NG
            ets = [None] * NG

            for it in range(NG + 6):
                # ---- x prefetch (pairs, even its) ----
                if it % 2 == 0 and it + PF < NG:
                    g0 = it + PF
                    sl = g0 % XR
                    nc.sync.dma_start(
                        x_ring[:, sl:sl + 2], xt_d[:, g0:g0 + 2])

                # ---- PE: projections for group a=it ----
                a = it
                if a < NG:
                    xa = x_ring[:, a % XR]
                    qk_h = qk[a % 2]
                    for cc in range(EC):
                        nc.tensor.matmul(
                            qk_h,
                            wqk_sb[:, cc, :],
                            xa[:, cc].rearrange("p t s -> p (t s)"),
                            start=(cc == 0),
                            stop=(cc == EC - 1),
                        )
                    ph = pair[a % 2]
                    for t in range(GRP):
                        for sb in range(2):
                            for cc in range(EC):
                                nc.tensor.matmul(
                                    ph[:, t, 384 + sb * H:384 + (sb + 1) * H],
                                    xa[:, cc, t, sb * 128:(sb + 1) * 128],
                                    wv_sb[:, cc, :],
                                    start=(cc == 0),
                                    stop=(cc == EC - 1),
                                )

                # ---- PE: scores for group c=it-2 ----
                c = it - 2
                if 0 <= c < NG:
                    ph = pair[c % 2]
                    QT, KT = qts[c], kts[c]
                    for t in range(GRP):
                        q0 = t * 256
                        nc.tensor.matmul(
                            ph[:, t, 0:128],
                            KT[:, q0 + 128:q0 + 256],
                            QT[:, q0 + 128:q0 + 256],
                            start=True, stop=True,
                        )
                        nc.tensor.matmul(
                            ph[:, t, 128:384],
                            KT[:, q0:q0 + 128],
                            QT[:, q0:q0 + 256],
                            start=True, stop=True,
                        )

                # ---- PE: AV for group f=it-5 ----
                f = it - 5
                if 0 <= f < NG:
                    et = ets[f]
                    avh = av[f % 2]
                    vs = v_sb[:, f % VS]
                    for t in range(GRP):
                        o0 = t * 2
                        nc.tensor.matmul(
                            avh[:, o0 * 128:o0 * 128 + 65],
                            et[:, t, 128:256],
                            vs[:, o0, :], start=True, stop=True,
                        )
                        nc.tensor.matmul(
                            avh[:, (o0 + 1) * 128:(o0 + 1) * 128 + 65],
                            et[:, t, 256:384],
                            vs[:, o0, :], start=True, stop=False,
                        )
                        nc.tensor.matmul(
                            avh[:, (o0 + 1) * 128:(o0 + 1) * 128 + 65],
                            et[:, t, 0:128],
                            vs[:, o0 + 1, :], start=False, stop=True,
                        )

                # ---- drains for group d=it-1 (DVE: qt then v; Act: kt) ----
                d = it - 1
                if 0 <= d < NG:
                    qk_h = qk[d % 2]
                    qt = qk_pool.tile([64, 512], BF16, tag="qt")
                    kt = qk_pool.tile([64, 512], BF16, tag="kt")
                    nc.vector.tensor_copy(qt, qk_h[0:64, :])
                    nc.scalar.activation(kt, qk_h[64:128, :], CPY)
                    nc.vector.tensor_copy(
                        v_sb[:, d % VS, :, 0:H].rearrange(
                            "p (t s) h -> p t s h", t=2),
                        pair[d % 2][:, :, 384:512].rearrange(
                            "p t (s h) -> p t s h", s=2),
                    )
                    qts[d], kts[d] = qt, kt

                # ---- Act: exp for group e=it-3 ----
                e = it - 3
                if 0 <= e < NG:
                    et = et_pool.tile([128, GRP, 384], BF16, tag="et")
                    nc.scalar.activation(
                        et, pair[e % 2][:, :, 0:384], EXP, scale=SCALE,
                    )
                    ets[e] = et

                # ---- Pool: causal mask for group m=it-4 ----
                m = it - 4
                if 0 <= m < NG:
                    et = ets[m]
                    nc.gpsimd.affine_select(
                        out=et[:, :, 0:256].rearrange(
                            "p t (d2 i) -> p t d2 i", d2=2),
                        in_=et[:, :, 0:256].rearrange(
                            "p t (d2 i) -> p t d2 i", d2=2),
                        compare_op=mybir.AluOpType.is_ge, fill=0.0,
                        base=0, pattern=[[0, GRP], [0, 2], [1, 128]],
                        channel_multiplier=-1,
                    )

                # ---- DVE: normalize for group n=it-6; out DMA (odd its) ----
                n = it - 6
                if 0 <= n < NG:
                    avh = av[n % 2].rearrange("p (j c) -> p j c", j=4)
                    ot = ot_ring[:, n % 8]
                    rc = et_pool.tile([128, GRP * 2], F32, tag="rc")
                    nc.vector.reciprocal_approx_fast(
                        out=rc, in_=avh[:, :, 64])
                    nc.vector.tensor_tensor(
                        ot, avh[:, :, 0:H],
                        rc.broadcast_to([128, GRP * 2, H]),
                        mybir.AluOpType.mult,
                    )
                    if n % 2 == 1:
                        sl = (n - 1) % 8
                        nc.sync.dma_start(
                            out_d[:, n - 1:n + 1],
                            ot_ring[:, sl:sl + 2].rearrange(
                                "p r a h -> p r (a h)"),
                        )

    nc.finalize()
    return nc


def _prep_consts(Wq, Wk, Wv):
    bf = ml_dtypes.bfloat16
    wqk = np.empty((128, EC, 128), dtype=bf)
    wv = np.empty((128, EC, H), dtype=bf)
    for c in range(EC):
        wqk[:, c, 0:H] = Wq[c * 128:(c + 1) * 128, :].astype(bf)
        wqk[:, c, H:128] = Wk[c * 128:(c + 1) * 128, :].astype(bf)
        wv[:, c, :] = Wv[c * 128:(c + 1) * 128, :].astype(bf)
    return wqk, wv


def _prep_x(x):
    xr = x.astype(ml_dtypes.bfloat16).reshape(NCORES, NG, GRP, S, EC, 128)
    return np.ascontiguousarray(xr.transpose(0, 5, 1, 4, 2, 3))


def _unprep_out(res):
    o = np.stack([r["out"] for r in res])
    o = o.reshape(NCORES, 128, NG, GRP, 2, H).transpose(0, 2, 3, 4, 1, 5)
    return np.ascontiguousarray(o).astype(np.float32).reshape(B, S, H)


def kernel(x, Wq, Wk, Wv):
    x = np.asarray(x, dtype=np.float32)
    wqk, wv = _prep_consts(
        np.asarray(Wq, np.float32), np.asarray(Wk, np.float32),
        np.asarray(Wv, np.float32),
    )
    if "nc" not in _cache:
        _cache["nc"] = build_nc()
    nc = _cache["nc"]

    xt = _prep_x(x)
    in_maps = [{"xt": xt[core], "wqk": wqk, "wv": wv} for core in range(NCORES)]
    res = run_bass_kernel_spmd(nc, in_maps, core_ids=list(range(NCORES)))
    return _unprep_out(res.results)
